# revision 12
# baseline (speedup 1.0000x reference)
"""Trainium2 Bass kernel for nn_DecoderRNN (teacher-forced LSTM decoder).

Computation (B=64, T=20, E=512, V=32000):
    emb = W_emb[captions]                        # [B, T, E] gather
    h0 = 0, c0 = features
    h_t = LSTM(emb[:, t-1], h_{t-1}, c_{t-1})    # t = 1..19
    seq = [emb[:, 0], h_1 .. h_19]               # [B, T, E]
    logits = seq @ W_out.T + b_out               # [B, T, V]

Sharding: LSTM replicated on all 8 cores (sequential, small); output
projection sharded over the vocab dim (4000 rows per core) — it dominates
FLOPs.  All matmuls run as float32r (TF32-like, full PE rate at N>=256).

Per-core kernel phases:
  A. indirect-DMA gather of the 1280 token embeddings + PE transposes
     into embT (e on partitions, tokens in (t, b) order on free dim)
  B. one big GEMM: xproj[token] = emb @ W_ihT + (b_ih + b_hh)
  C. 19 LSTM steps.  Per step: inject xproj via a 64x64 identity matmul
     into the gates PSUM group, accumulate h @ W_hhT on top, activations
     on ScalarE, c/h updates on VectorE, PE-transpose h into seqT.
  D. projection: out[(t,b), v_shard] = seqT.T @ W_outT + b_out, streamed
     in two 2000-column halves of the vocab shard.
"""

import numpy as np

B = 64
T = 20
E = 512
V = 32000
G = 4 * E           # 2048
NCORES = 8
VSH = V // NCORES   # 4000 vocab rows per core
NTOK = B * T        # 1280 tokens, (t, b) order
NKE = E // 128      # 4 contraction tiles over E

_RUNNER_CACHE = {}


def _build_bass():
    import os
    import concourse.bacc as bacc
    import concourse.mybir as mybir
    from concourse.tile import TileContext
    from concourse.bass import IndirectOffsetOnAxis
    from concourse.masks import make_identity

    debug = bool(int(os.environ.get("DECODER_KERNEL_DEBUG", "0")))

    f32 = mybir.dt.float32
    f32r = mybir.dt.float32r
    Alu = mybir.AluOpType
    Act = mybir.ActivationFunctionType

    nc = bacc.Bacc()

    feat = nc.declare_dram_parameter("feat", [B, E], f32, isOutput=False)
    w_emb = nc.declare_dram_parameter("w_emb", [V, E], f32, isOutput=False)
    idx_in = nc.declare_dram_parameter("idx_in", [128, NTOK // 128], mybir.dt.int32, isOutput=False)
    w_ihT = nc.declare_dram_parameter("w_ihT", [E, G], f32, isOutput=False)
    w_hhT = nc.declare_dram_parameter("w_hhT", [E, G], f32, isOutput=False)
    bias_g = nc.declare_dram_parameter("bias_g", [128, G], f32, isOutput=False)
    e2 = nc.declare_dram_parameter("e2", [128, 128], f32, isOutput=False)
    w_outT = nc.declare_dram_parameter("w_outT", [E, VSH], f32, isOutput=False)
    b_outR = nc.declare_dram_parameter("b_outR", [128, VSH], f32, isOutput=False)
    out = nc.declare_dram_parameter("out", [NTOK, VSH], f32, isOutput=True)
    if debug:
        dbg_xp0 = nc.declare_dram_parameter("dbg_xp0", [128, G], f32, isOutput=True)
        dbg_seqT = nc.declare_dram_parameter("dbg_seqT", [128, NKE * NTOK], f32, isOutput=True)

    NG = NTOK // 128            # 10 gather tiles / xproj m-tiles

    with TileContext(nc) as tc:
      with tc.tile_pool(name="seqT", bufs=1) as seqT_pool:
        # seqT: [128, 4*1280], k-th E-tile occupies cols [k*1280, (k+1)*1280)
        seqT = seqT_pool.tile([128, NKE * NTOK], f32r, tag="seqT")
        with tc.tile_pool(name="const", bufs=1) as cpool, \
             tc.tile_pool(name="whh", bufs=1) as whh_pool, \
             tc.tile_pool(name="xproj", bufs=1) as xp_pool, \
             tc.tile_pool(name="act", bufs=1) as act_pool, \
             tc.tile_pool(name="state", bufs=1) as state_pool, \
             tc.tile_pool(name="psumBC", bufs=1, space="PSUM") as psbc:

            # --- constants / weights ---
            idx_t = cpool.tile([128, NG], mybir.dt.int32, tag="idx")
            nc.sync.dma_start(idx_t[:], idx_in[:])
            iden = cpool.tile([128, 128], f32, tag="iden")
            make_identity(nc, iden[:])
            e2r = cpool.tile([128, 128], f32r, tag="e2r")
            nc.sync.dma_start(e2r[:], e2[:].bitcast(f32r))

            whh_t = whh_pool.tile([128, NKE * G], f32r, tag="whh")
            nc.sync.dma_start(
                whh_t[:].rearrange("p (k g) -> p k g", k=NKE),
                w_hhT[:].bitcast(f32r).rearrange("(k p) g -> p k g", p=128))

            # c state (c0 = features)
            c_t = state_pool.tile([B, E], f32, tag="c")
            nc.sync.dma_start(c_t[:], feat[:])

            xp_tiles = []

            # --- phases A+B interleaved: per gather tile, transpose then
            # immediately run its x-projection GEMM (stationary = embt_g) ---
            with tc.tile_pool(name="wih", bufs=1) as wih_pool, \
                 tc.tile_pool(name="biasp", bufs=1) as bias_pool, \
                 tc.tile_pool(name="gather", bufs=2) as gat_pool, \
                 tc.tile_pool(name="embt", bufs=2) as embt_pool:
                bias_t = bias_pool.tile([128, G], f32, tag="biasg")
                nc.sync.dma_start(bias_t[:], bias_g[:])
                wih_t = wih_pool.tile([128, NKE * G], f32r, tag="wih")
                nc.sync.dma_start(
                    wih_t[:].rearrange("p (k g) -> p k g", k=NKE),
                    w_ihT[:].bitcast(f32r).rearrange("(k p) g -> p k g", p=128))

                for g in range(NG):
                    gt = gat_pool.tile([128, E], f32, tag="gt")
                    nc.gpsimd.indirect_dma_start(
                        out=gt[:], out_offset=None,
                        in_=w_emb[:],
                        in_offset=IndirectOffsetOnAxis(ap=idx_t[:, g:g + 1], axis=0))
                    embt_g = embt_pool.tile([128, NKE * 128], f32r, tag="embt")
                    for k in range(NKE):
                        tp = psbc.tile([128, 128], f32, tag="tr")
                        nc.tensor.transpose(tp[:], gt[:, k * 128:(k + 1) * 128], iden[:])
                        nc.vector.tensor_copy(embt_g[:, k * 128:(k + 1) * 128], tp[:])
                    if g == 0:
                        # seqT block 0 = emb of captions[:, 0]
                        for k in range(NKE):
                            nc.vector.tensor_copy(
                                seqT[:, k * NTOK: k * NTOK + B],
                                embt_g[:, k * 128: k * 128 + B])
                    xp = xp_pool.tile([128, G], f32r, tag=f"xp{g}")
                    xp_tiles.append(xp)
                    for n in range(4):
                        ps = psbc.tile([128, 512], mybir.dt.float32, tag="xg")
                        for k in range(NKE):
                            nc.tensor.matmul(
                                ps[:],
                                embt_g[:, k * 128:(k + 1) * 128],
                                wih_t[:, k * G + n * 512: k * G + (n + 1) * 512],
                                start=(k == 0), stop=(k == NKE - 1))
                        nc.vector.tensor_tensor(
                            out=xp[:, n * 512:(n + 1) * 512], in0=ps[:],
                            in1=bias_t[:, n * 512:(n + 1) * 512], op=Alu.add)

            if debug:
                dxp = act_pool.tile([128, G], f32, tag="dbgxp")
                nc.vector.tensor_copy(dxp[:], xp_tiles[0][:])
                nc.sync.dma_start(dbg_xp0[:], dxp[:])

            # --- phase C: LSTM steps ---
            for t in range(1, T):
                m, half = (t - 1) // 2, (t - 1) % 2
                po = half * 64
                xp = xp_tiles[m]
                gp = psbc.tile([B, G], mybir.dt.float32, tag="gates")
                only_inject = (t == 1)
                for n in range(4):
                    nc.tensor.matmul(
                        gp[:, n * 512:(n + 1) * 512],
                        e2r[po:po + 64, po:po + 64],
                        xp[po:po + 64, n * 512:(n + 1) * 512],
                        start=True, stop=only_inject)
                if not only_inject:
                    for k in range(NKE):
                        for n in range(4):
                            nc.tensor.matmul(
                                gp[:, n * 512:(n + 1) * 512],
                                seqT[:, k * NTOK + (t - 1) * 64: k * NTOK + t * 64],
                                whh_t[:, k * G + n * 512: k * G + (n + 1) * 512],
                                start=False, stop=(k == NKE - 1))
                # gate order (g, i, f, o) after host-side weight permutation
                g_t = act_pool.tile([B, E], f32, tag="gg")
                i_t = act_pool.tile([B, E], f32, tag="gi")
                f_t = act_pool.tile([B, E], f32, tag="gf")
                o_t = act_pool.tile([B, E], f32, tag="go")
                nc.scalar.activation(g_t[:], gp[:, 0:512], Act.Tanh)
                nc.scalar.activation(i_t[:], gp[:, 512:1024], Act.Sigmoid)
                nc.scalar.activation(f_t[:], gp[:, 1024:1536], Act.Sigmoid)
                nc.scalar.activation(o_t[:], gp[:, 1536:2048], Act.Sigmoid)
                tmp = act_pool.tile([B, E], f32, tag="tmp")
                nc.vector.tensor_tensor(out=tmp[:], in0=i_t[:], in1=g_t[:], op=Alu.mult)
                nc.vector.tensor_tensor(out=c_t[:], in0=f_t[:], in1=c_t[:], op=Alu.mult)
                nc.vector.tensor_tensor(out=c_t[:], in0=c_t[:], in1=tmp[:], op=Alu.add)
                tc_t = act_pool.tile([B, E], f32, tag="tc")
                nc.scalar.activation(tc_t[:], c_t[:], Act.Tanh)
                h_t = act_pool.tile([B, E], f32, tag="h")
                nc.vector.tensor_tensor(out=h_t[:], in0=o_t[:], in1=tc_t[:], op=Alu.mult)
                for k in range(NKE):
                    tp = psbc.tile([128, 64], f32, tag="tr")
                    nc.tensor.transpose(tp[:], h_t[:, k * 128:(k + 1) * 128],
                                        iden[0:64, 0:64])
                    nc.vector.tensor_copy(
                        seqT[:, k * NTOK + t * 64: k * NTOK + (t + 1) * 64], tp[:])

        if debug:
            with tc.tile_pool(name="dbgseq", bufs=1) as dbg_pool:
                ds_t = dbg_pool.tile([128, NKE * NTOK], f32, tag="dbgseq")
                nc.vector.tensor_copy(ds_t[:], seqT[:])
                nc.sync.dma_start(dbg_seqT[:], ds_t[:])

        # --- phase D: vocab projection (separate pools; earlier SBUF freed) ---
        with tc.tile_pool(name="wout", bufs=1) as wout_pool, \
             tc.tile_pool(name="outb", bufs=2) as out_pool, \
             tc.tile_pool(name="bout", bufs=1) as bout_pool, \
             tc.tile_pool(name="psumD", bufs=2, space="PSUM") as psd:
            bout_t = bout_pool.tile([128, VSH], f32, tag="bout")
            nc.sync.dma_start(bout_t[:], b_outR[:])
            HALF = VSH // 2  # 2000
            for half in range(2):
                wh = wout_pool.tile([128, NKE * HALF], f32r, tag=f"wh{half}")
                nc.sync.dma_start(
                    wh[:].rearrange("p (k v) -> p k v", k=NKE),
                    w_outT[:, half * HALF:(half + 1) * HALF]
                    .bitcast(f32r).rearrange("(k p) v -> p k v", p=128))
                for m in range(NG):
                    # 4 x 500-wide slices, each starting on a 512 (bank) boundary
                    ps = psd.tile([128, 4 * 512], mybir.dt.float32, tag="proj")
                    for k in range(NKE):
                        for n4 in range(4):
                            nc.tensor.matmul(
                                ps[:, n4 * 512: n4 * 512 + 500],
                                seqT[:, k * NTOK + m * 128: k * NTOK + (m + 1) * 128],
                                wh[:, k * HALF + n4 * 500: k * HALF + (n4 + 1) * 500],
                                start=(k == 0), stop=(k == NKE - 1))
                    ob = out_pool.tile([128, HALF], f32, tag="ob")
                    nc.vector.tensor_tensor(
                        out=ob[:].rearrange("p (n c) -> p n c", c=500),
                        in0=ps[:].rearrange("p (n c) -> p n c", c=512)[:, :, 0:500],
                        in1=bout_t[:, half * HALF:(half + 1) * HALF]
                        .rearrange("p (n c) -> p n c", c=500),
                        op=Alu.add)
                    nc.sync.dma_start(
                        out[m * 128:(m + 1) * 128, half * HALF:(half + 1) * HALF],
                        ob[:])

    nc.compile()
    return nc


def _get_runner():
    """Build + compile the SPMD program once; return a reusable executor."""
    if "runner" in _RUNNER_CACHE:
        return _RUNNER_CACHE["runner"]

    import jax
    import numpy as np
    from jax.sharding import Mesh, PartitionSpec
    from jax.experimental.shard_map import shard_map
    import concourse.mybir as mybir
    from concourse import bass2jax

    nc = _build_bass()

    bass2jax.install_neuronx_cc_hook()

    partition_name = nc.partition_id_tensor.name if nc.partition_id_tensor else None
    in_names, out_names, out_avals, zero_shapes = [], [], [], []
    for alloc in nc.m.functions[0].allocations:
        if not isinstance(alloc, mybir.MemoryLocationSet):
            continue
        name = alloc.memorylocations[0].name
        if alloc.kind == "ExternalInput":
            if name != partition_name:
                in_names.append(name)
        elif alloc.kind == "ExternalOutput":
            shape = tuple(alloc.tensor_shape)
            dtype = mybir.dt.np(alloc.dtype)
            out_names.append(name)
            out_avals.append(jax.core.ShapedArray(shape, dtype))
            zero_shapes.append((shape, dtype))
    n_params = len(in_names)
    n_outs = len(out_avals)
    all_in_names = list(in_names) + list(out_names)
    if partition_name is not None:
        all_in_names.append(partition_name)

    def _body(*args):
        operands = list(args)
        if partition_name is not None:
            operands.append(bass2jax.partition_id_tensor())
        outs = bass2jax._bass_exec_p.bind(
            *operands,
            out_avals=tuple(out_avals),
            in_names=tuple(all_in_names),
            out_names=tuple(out_names),
            lowering_input_output_aliases=(),
            sim_require_finite=True,
            sim_require_nnan=True,
            nc=nc,
        )
        return tuple(outs)

    donate = tuple(range(n_params, n_params + n_outs))
    devices = jax.devices()[:NCORES]
    mesh = Mesh(np.asarray(devices), ("core",))
    in_specs = (PartitionSpec("core"),) * (n_params + n_outs)
    out_specs = (PartitionSpec("core"),) * n_outs
    sharded = jax.jit(
        shard_map(_body, mesh=mesh, in_specs=in_specs, out_specs=out_specs,
                  check_rep=False),
        donate_argnums=donate, keep_unused=True)

    class Runner:
        def __init__(self):
            self.sharded = sharded
            self.in_names = in_names
            self.out_names = out_names
            self.zero_shapes = zero_shapes
            self.n_params = n_params
            self.mesh = mesh

        def concat_inputs(self, in_maps):
            return [
                np.concatenate([np.asarray(in_maps[c][nm]) for c in range(NCORES)], axis=0)
                for nm in self.in_names
            ]

        def zeros(self):
            return [np.zeros((NCORES * s[0],) + tuple(s[1:]), d)
                    for (s, d) in self.zero_shapes]

        def __call__(self, in_maps):
            concat_in = self.concat_inputs(in_maps)
            out_arrs = self.sharded(*concat_in, *self.zeros())
            return {
                nm: np.asarray(out_arrs[i]).reshape(
                    (NCORES,) + tuple(self.zero_shapes[i][0]))
                for i, nm in enumerate(self.out_names)
            }

    r = Runner()
    _RUNNER_CACHE["runner"] = r
    return r


def _prep_inputs(features, captions, W_emb, W_out, b_out, W_ih, W_hh, b_ih, b_hh):
    """Host-side shard/replicate/transpose prep.  Gate order is permuted
    from PyTorch (i, f, g, o) to kernel order (g, i, f, o)."""
    features = np.asarray(features, dtype=np.float32)
    captions = np.asarray(captions).astype(np.int32)
    W_emb = np.ascontiguousarray(np.asarray(W_emb, dtype=np.float32))
    W_out = np.asarray(W_out, dtype=np.float32)
    b_out = np.asarray(b_out, dtype=np.float32)
    W_ih = np.asarray(W_ih, dtype=np.float32)
    W_hh = np.asarray(W_hh, dtype=np.float32)
    b_ih = np.asarray(b_ih, dtype=np.float32)
    b_hh = np.asarray(b_hh, dtype=np.float32)

    perm = np.concatenate([np.arange(2 * E, 3 * E),     # g
                           np.arange(0, E),             # i
                           np.arange(E, 2 * E),         # f
                           np.arange(3 * E, 4 * E)])    # o
    w_ihT = np.ascontiguousarray(W_ih[perm].T)          # [E, G]
    w_hhT = np.ascontiguousarray(W_hh[perm].T)          # [E, G]
    bias = (b_ih + b_hh)[perm].astype(np.float32)
    bias_g = np.ascontiguousarray(np.broadcast_to(bias[None, :], (128, G)))

    cap_seq = np.ascontiguousarray(captions.T).reshape(-1)          # (t, b) order, [1280]
    idx_in = np.ascontiguousarray(cap_seq.reshape(NTOK // 128, 128).T).astype(np.int32)

    e2 = np.zeros((128, 128), np.float32)
    e2[0:64, 0:64] = np.eye(64, dtype=np.float32)
    e2[64:128, 64:128] = np.eye(64, dtype=np.float32)

    in_maps = []
    for c in range(NCORES):
        wsh = W_out[c * VSH:(c + 1) * VSH]                           # [4000, E]
        w_outT = np.ascontiguousarray(wsh.T)                         # [E, 4000]
        b_outR = np.ascontiguousarray(
            np.broadcast_to(b_out[c * VSH:(c + 1) * VSH][None, :], (128, VSH)))
        in_maps.append({
            "feat": features, "w_emb": W_emb, "idx_in": idx_in,
            "w_ihT": w_ihT, "w_hhT": w_hhT, "bias_g": bias_g, "e2": e2,
            "w_outT": w_outT, "b_outR": b_outR,
        })
    return in_maps


def kernel(features, captions, W_emb, W_out, b_out, W_ih, W_hh, b_ih, b_hh):
    runner = _get_runner()
    in_maps = _prep_inputs(features, captions, W_emb, W_out, b_out,
                           W_ih, W_hh, b_ih, b_hh)
    res = runner(in_maps)["out"]               # [8, 1280, 4000]
    full = np.concatenate(list(res), axis=1)   # [1280, 32000]
    logits = full.reshape(T, B, V).transpose(1, 0, 2)  # [B, T, V]
    return np.ascontiguousarray(logits)
